# revision 16
# baseline (speedup 1.0000x reference)
"""FLAMETex kernel for Trainium2 (8 NeuronCores, Bass/Tile).

Reference computes tex = mean + basis @ texcode^T over the FULL 786432-row
texture, then downsamples 2x, flips channels (BGR), and gathers 5023 UV
points.  Only 3*5023 = 15069 texture rows can ever reach the output, and
the row indices depend only on uv_coords (an input).  So: compute the
gather indices on the host, gather the needed basis/mean rows, and run a
small (15104 x 201) @ (201 x 8) GEMM on device, row-sharded over the 8
cores (1888 rows each: 14 m-tiles of 128 + one of 96).

Per-core device layout: blob (201, 1896) f32 in DRAM; cols 0:8 hold
[texcode | ones]^T (mean folded in as the 201st contraction row), cols 8:
hold the gathered [basis | mean]^T shard.  The GEMM runs with the basis
slices as the STATIONARY operand (M<=128, full PE array) and the 8-column
x operand MOVING: per m-tile, the two contraction chunks (rows 0:128 /
73 rows 128:201) run as a back-to-back start/stop accumulation pair into
the tile's 8-column slice of a single (128, 120) PSUM bank -- at most one
open accumulation group per bank, which hardware requires (group state is
bank-granular; a two-pass all-c0-then-all-c1 order returns wrong data).
One DVE copy drains the bank; one DMA writes out_c (128, 120) = R-shard
in (tile, row)-interleaved layout that the host untangles.

Perf structure (TimelineSim-guided, 28.4us -> 10.9us/core):
 - chunk-0 column pieces (512,512,512,352) stream on the sync-engine
   HWDGE; chunk-1 goes through gpsimd/SWDGE in 4 pieces so the two DGE
   paths run in parallel;
 - five tiny "hold" matmuls in front wait on the first DMA and fill the
   PE sequencer's run-ahead window, so every real matmul is costed after
   ~3.3us (full p-state tier); on hardware they are 27ns each;
 - fp32 throughout (fp32r measured at ~1.5e-4 rel err - too coarse for
   an fp32-envelope gate; fp32 gives ~7e-8).
"""

import hashlib
import os
import shutil

import numpy as np

import concourse.bacc as bacc
import concourse.bass2jax as bass2jax
import concourse.mybir as mybir
import concourse.tile as tile
from concourse.bass_utils import run_bass_kernel_spmd

B = 8
K = 200
N_UV = 5023
V = 786432
ROWS = 3 * N_UV          # 15069 gathered texture rows
N_CORES = 8
PER_CORE = 1888          # 14 m-tiles of 128 + one of 96; 8 * 1888 = 15104 >= 15069
ROWS_PAD = N_CORES * PER_CORE
KA = K + 1               # contraction with the mean folded in
KC = 128                 # first contraction chunk (partition dim)
KC1 = KA - KC            # 73 rows in the second chunk
AW = B + PER_CORE        # blob width
MT = 128                 # m-tile height (PSUM partitions)
MT_HEIGHTS = (MT,) * 14 + (96,)
NMT = len(MT_HEIGHTS)    # 15
C0_GROUPS = (512, 512, 512, 352)
N_C1 = 4
N_HOLD = 5

_NC_CACHE = {}
_NEFF_CACHE_ROOT = "/tmp/bass_neff_cache"


def _install_neff_cache():
    """Cache compiled NEFFs by BIR content hash across processes.

    The bass2jax neuronx_cc_hook recompiles the identical BIR (a multi-
    minute walrus run with birsim enabled) on every fresh process. The
    kernel's BIR serialization is deterministic, so a sha256-keyed copy of
    the NEFF makes repeat cold starts ~2s instead of minutes. Falls back
    to the original compile on any cache error.
    """
    if getattr(bass2jax, "_flametex_neff_cache", False):
        return
    orig = getattr(bass2jax, "compile_bir_kernel", None)
    if orig is None:
        return

    def cached(bir_json, tmpdir, neff_name="file.neff"):
        key = hashlib.sha256(bir_json).hexdigest()
        cpath = os.path.join(_NEFF_CACHE_ROOT, key, "file.neff")
        dst = os.path.join(tmpdir, neff_name)
        try:
            if os.path.exists(cpath):
                shutil.copy(cpath, dst)
                return dst
        except OSError:
            pass
        neff = orig(bir_json, tmpdir, neff_name=neff_name)
        try:
            os.makedirs(os.path.dirname(cpath), exist_ok=True)
            tmp = cpath + f".tmp{os.getpid()}"
            shutil.copy(neff, tmp)
            os.replace(tmp, cpath)
        except OSError:
            pass
        return neff

    bass2jax.compile_bir_kernel = cached
    bass2jax._flametex_neff_cache = True


def _build_nc():
    if "nc" in _NC_CACHE:
        return _NC_CACHE["nc"]
    f32 = mybir.dt.float32
    nc = bacc.Bacc("TRN2")
    blob = nc.dram_tensor("blob", (KA, AW), f32, kind="ExternalInput")
    out_c = nc.dram_tensor("out_c", (MT, NMT * B), f32, kind="ExternalOutput")
    NT = len(C0_GROUPS)
    starts = [B + sum(C0_GROUPS[:j]) for j in range(NT)]

    with tile.TileContext(nc) as tc:
        with (
            tc.tile_pool(name="ap", bufs=1) as ap,
            tc.tile_pool(name="op", bufs=1) as op,
            tc.tile_pool(name="pp", bufs=1, space="PSUM") as pp,
        ):
            a = ap.tile([KC, 2 * AW], f32, tag="a")
            a3 = a[:, :].rearrange("p (c w) -> p c w", c=2)

            g0w = B + C0_GROUPS[0]
            nc.sync.dma_start(a3[0:KC, 0, 0:g0w], blob[0:KC, 0:g0w])
            for j in range(1, NT):
                lo = starts[j]
                nc.sync.dma_start(
                    a3[0:KC, 0, lo : lo + C0_GROUPS[j]],
                    blob[0:KC, lo : lo + C0_GROUPS[j]],
                )
            step = AW // N_C1
            cuts = [0] + [step * i for i in range(1, N_C1)] + [AW]
            for i in range(N_C1):
                nc.gpsimd.dma_start(
                    a3[0:KC1, 1, cuts[i] : cuts[i + 1]],
                    blob[KC:KA, cuts[i] : cuts[i + 1]],
                )

            hps = pp.tile([B, 512], f32, tag="hold")
            for _ in range(N_HOLD):
                nc.tensor.matmul(
                    hps[:, 0:8], a3[:, 0, 0:B], a3[:, 0, B : B + 8],
                    start=True, stop=True,
                )

            # one open accumulation group at a time: HW PSUM group state is
            # bank-granular, so the c0/c1 pair for each m-tile must close
            # before the next tile's pair opens
            ps = pp.tile([MT, NMT * B], f32, tag="ps")
            lo = B
            for mt, mh in enumerate(MT_HEIGHTS):
                nc.tensor.matmul(
                    ps[0:mh, mt * B : (mt + 1) * B],
                    a3[:, 0, lo : lo + mh],
                    a3[:, 0, 0:B],
                    start=True,
                    stop=False,
                )
                nc.tensor.matmul(
                    ps[0:mh, mt * B : (mt + 1) * B],
                    a3[0:KC1, 1, lo : lo + mh],
                    a3[0:KC1, 1, 0:B],
                    start=False,
                    stop=True,
                )
                lo += mh

            ot = op.tile([MT, NMT * B], f32, tag="ot")
            nc.vector.tensor_copy(ot[:, :], ps[:, :])
            nc.sync.dma_start(out_c[:, :], ot[:, :])

    nc.finalize()
    _NC_CACHE["nc"] = nc
    return nc


def kernel(texcode, uv_coords, texture_mean, texture_basis):
    texcode = np.asarray(texcode, dtype=np.float32)
    uv = np.asarray(uv_coords, dtype=np.float32)
    mean = np.asarray(texture_mean, dtype=np.float32).reshape(V)
    basis = np.asarray(texture_basis, dtype=np.float32).reshape(V, K)

    # replicate reference index math exactly in float32
    x = np.clip((uv[:, 0] * np.float32(256.0)).astype(np.int32), 0, 255)
    y = np.clip(
        ((np.float32(1.0) - uv[:, 1]) * np.float32(256.0)).astype(np.int32), 0, 255
    )
    # flat index into the (786432,) texture for output row r = n*3 + c:
    #   v = (2y)*512*3 + (2x)*3 + (2 - c)
    base = 3072 * y.astype(np.int64) + 6 * x.astype(np.int64)
    vidx = (base[:, None] + np.array([2, 1, 0], dtype=np.int64)[None, :]).reshape(-1)

    at = np.zeros((KA, ROWS_PAD), dtype=np.float32)
    at[:K, :ROWS] = basis[vidx].T
    at[K, :ROWS] = mean[vidx]
    xt = np.empty((KA, B), dtype=np.float32)
    xt[:K, :] = texcode.T
    xt[K, :] = 1.0

    _install_neff_cache()
    nc = _build_nc()
    in_maps = []
    for i in range(N_CORES):
        blob = np.empty((KA, AW), dtype=np.float32)
        blob[:, :B] = xt
        blob[:, B:] = at[:, i * PER_CORE : (i + 1) * PER_CORE]
        in_maps.append({"blob": blob})
    res = run_bass_kernel_spmd(nc, in_maps, core_ids=list(range(N_CORES)))

    # out_c[core][p, mt*8 + b] = R[core*1888 + sum(heights[:mt]) + p, b]
    r_parts = []
    for r in res.results:
        arr = r["out_c"].reshape(MT, NMT, B).transpose(1, 0, 2)  # (tile, row, b)
        r_parts.append(
            np.concatenate(
                [arr[:-1].reshape((NMT - 1) * MT, B), arr[-1, : MT_HEIGHTS[-1]]]
            )
        )
    r_full = np.concatenate(r_parts, axis=0)[:ROWS]  # (15069, 8)
    out = r_full.reshape(N_UV, 3, B).transpose(2, 1, 0)  # (B, 3, N_UV)
    return np.ascontiguousarray(out)
